# revision 17
# baseline (speedup 1.0000x reference)
"""AIF (attention-with-integrate-and-fire) sparse attention kernel for 8 TRN2 cores.

Strategy: data-parallel over batch (B=16 -> 2 batches/core, no collectives).
Per batch the AIF mask is a per-query prefix span: query i attends frames t with
csum[t] < i+1 (csum monotone) so only a short prefix (~2*ys_len <= ~512 frames)
of the 2048 encoder frames ever receives attention.  The kernel:
  - streams all xs once (f32) computing alphas (DVE dot), xs column-sum
    (f32r matmul) and the masked cumsum (PE transpose + DVE scan),
  - projects only a static 768-frame prefix window through W_proj in bf16,
  - computes masked softmax attention over that window (penalty mask from
    broadcast csum vs per-partition query thresholds, exp+rowsum fused on ACT),
  - fills queries i >= ys_len with the uniform-attention fallback row
    mean_t(xs_proj) = (sum_t xs / T) @ W_proj + b_proj via an on-device select.
A host-side safety net recomputes (numpy) any batch whose true attended prefix
exceeds the 768-frame window — impossible for the reference input distribution
(would need mean(sigmoid) < 1/3 over 768 frames) but keeps the kernel correct
for arbitrary inputs.
"""

import os
import sys
from contextlib import ExitStack

import numpy as np

sys.path.insert(0, "/opt/trn_rl_repo")
sys.path.insert(0, "/opt/trn_rl_repo/concourse")

import concourse.bass as bass
import concourse.tile as tile
from concourse import mybir
from concourse.bass_utils import run_bass_kernel_spmd

B, T, V = 16, 2048, 256
D = 1024          # ENC_D == PRED_D
NB = 2            # batches per core
NCORES = 8
W = 768           # static attended-prefix window (frames)
NCHUNK = T // 256   # 8 streaming chunks of 256 frames
PREFIX_CHUNKS = W // 256  # 3
SCALE = 1.0 / np.sqrt(D)
F32 = mybir.dt.float32
F32R = mybir.dt.float32r
BF16 = mybir.dt.bfloat16
AF = mybir.ActivationFunctionType
OP = mybir.AluOpType


def _build_graph():
    nc = bass.Bass()

    xs = nc.declare_dram_parameter("xs", [NB, T, D], F32, isOutput=False)
    ys = nc.declare_dram_parameter("ys", [NB, V, D], F32, isOutput=False)
    w_out = nc.declare_dram_parameter("w_out", [D, 1], F32, isOutput=False)
    b_out = nc.declare_dram_parameter("b_out", [1], F32, isOutput=False)
    w_proj = nc.declare_dram_parameter("w_proj", [D, D], F32, isOutput=False)
    b_proj = nc.declare_dram_parameter("b_proj", [D], F32, isOutput=False)
    lens_f = nc.declare_dram_parameter("lens_f", [NB, 2], F32, isOutput=False)  # [xs_len, ys_len]
    iota_t = nc.declare_dram_parameter("iota_t", [128, 16], F32, isOutput=False)    # t = c*128+p
    iota_row = nc.declare_dram_parameter("iota_row", [1, T], F32, isOutput=False)
    thr_q = nc.declare_dram_parameter("thr_q", [128, 2], F32, isOutput=False)       # i+1
    iota_q = nc.declare_dram_parameter("iota_q", [128, 2], F32, isOutput=False)     # i
    ident = nc.declare_dram_parameter("ident", [128, 128], F32, isOutput=False)
    onesc = nc.declare_dram_parameter("onesc", [128, 1], F32, isOutput=False)       # ones column

    mrow_dram = nc.dram_tensor("mrow_dram", [NB, W], F32)
    fb_dram = nc.dram_tensor("fb_dram", [NB, D], F32)

    o_emb = nc.declare_dram_parameter("o_emb", [NB, V, D], F32, isOutput=True)
    o_alpha = nc.declare_dram_parameter("o_alpha", [NB, T], F32, isOutput=True)
    o_csum = nc.declare_dram_parameter("o_csum", [NB, T], F32, isOutput=True)
    o_tok = nc.declare_dram_parameter("o_tok", [NB, 1], F32, isOutput=True)

    with tile.TileContext(nc) as tc, ExitStack() as ctx:
        singles = ctx.enter_context(tc.tile_pool(name="singles", bufs=1))
        xfp = ctx.enter_context(tc.tile_pool(name="xfp", bufs=3))
        b16p = ctx.enter_context(tc.tile_pool(name="b16p", bufs=2))
        xtp = ctx.enter_context(tc.tile_pool(name="xtp", bufs=1))
        perb = ctx.enter_context(tc.tile_pool(name="perb", bufs=1))
        small = ctx.enter_context(tc.tile_pool(name="small", bufs=4))
        scr = ctx.enter_context(tc.tile_pool(name="scr", bufs=1))
        rows = ctx.enter_context(tc.tile_pool(name="rows", bufs=2))
        outp = ctx.enter_context(tc.tile_pool(name="outp", bufs=2))
        psA = ctx.enter_context(tc.tile_pool(name="psA", bufs=2, space="PSUM"))
        psPV = ctx.enter_context(tc.tile_pool(name="psPV", bufs=2, space="PSUM"))
        psX = ctx.enter_context(tc.tile_pool(name="psX", bufs=1, space="PSUM"))
        psT = ctx.enter_context(tc.tile_pool(name="psT", bufs=1, space="PSUM"))

        # ---- constants / weights staged once ----
        wb = singles.tile([128, D], F32)          # W_out broadcast to all partitions
        nc.gpsimd.dma_start(out=wb, in_=w_out[:, 0].partition_broadcast(128))
        bob = singles.tile([128, 1], F32)         # b_out broadcast
        nc.gpsimd.dma_start(out=bob, in_=b_out[0:1].partition_broadcast(128))
        bpT = singles.tile([128, 8], F32)         # b_proj as [e%128, e//128]
        nc.gpsimd.dma_start(out=bpT, in_=bass.AP(
            tensor=b_proj.ap().tensor, offset=0, ap=[[1, 128], [128, 8]]))
        bp_row = rows.tile([1, D], F32, tag="rows")
        nc.gpsimd.dma_start(out=bp_row, in_=b_proj[None, :])
        bp_row16 = singles.tile([1, D], BF16)
        nc.scalar.activation(out=bp_row16, in_=bp_row, func=AF.Copy)
        one11_16 = singles.tile([1, 1], BF16)
        nc.vector.memset(one11_16, 1.0)
        one11_f = singles.tile([1, 1], F32)
        nc.vector.memset(one11_f, 1.0)

        it_t = singles.tile([128, 16], F32)
        nc.gpsimd.dma_start(out=it_t, in_=iota_t[:, :])
        it_row = singles.tile([1, T], F32)
        nc.gpsimd.dma_start(out=it_row, in_=iota_row[:, :])
        t_thr = singles.tile([128, 2], F32)
        nc.gpsimd.dma_start(out=t_thr, in_=thr_q[:, :])
        t_ioq = singles.tile([128, 2], F32)
        nc.gpsimd.dma_start(out=t_ioq, in_=iota_q[:, :])
        idn = singles.tile([128, 128], F32)
        nc.gpsimd.dma_start(out=idn, in_=ident[:, :])
        ones16 = singles.tile([128, 1], BF16)
        nc.vector.memset(ones16, 1.0)

        # W_proj: load f32 slabs through the xf ring, cast to resident bf16
        wp16 = singles.tile([128, 8, D], BF16)    # [d%128, d//128, e]
        for k in range(8):
            wslab = xfp.tile([128, D], F32, tag="xf")
            nc.sync.dma_start(out=wslab, in_=w_proj[k * 128:(k + 1) * 128, :])
            nc.scalar.activation(out=wp16[:, k, :], in_=wslab, func=AF.Copy)

        # per-batch broadcast of lens (128,1) plus (1,1) scalars
        xlen_bc, ylen_bc, xlen_s = [], [], []
        for b in range(NB):
            xb = small.tile([128, 1], F32, tag="lxb")
            nc.gpsimd.dma_start(
                out=xb, in_=lens_f[b, 0:1].partition_broadcast(128))
            yb = small.tile([128, 1], F32, tag="lyb")
            nc.gpsimd.dma_start(
                out=yb, in_=lens_f[b, 1:2].partition_broadcast(128))
            xs_ = small.tile([1, 1], F32, tag="lxs")
            nc.gpsimd.dma_start(out=xs_, in_=lens_f[b:b + 1, 0:1])
            xlen_bc.append(xb)
            ylen_bc.append(yb)
            xlen_s.append(xs_)

        prod = scr.tile([128, D], BF16)           # dummy full-product output

        for b in range(NB):
            alog = small.tile([128, 16], F32, tag="alog")
            ps_xsum = psX.tile([1, D], F32, tag="xsum")
            # transposed bf16 xs prefix, per 512/256 group
            xsT0 = xtp.tile([128, 4, 8, 128], BF16, tag="xsT0")
            xsT1 = xtp.tile([128, 2, 8, 128], BF16, tag="xsT1")
            xsT = [xsT0, xsT1]

            # ---------- phase A: stream xs ----------
            for c in range(NCHUNK):
                xf = xfp.tile([128, 2, D], F32, tag="xf")
                nc.sync.dma_start(
                    out=xf,
                    in_=xs[b, c * 256:(c + 1) * 256, :].rearrange(
                        "(u p) d -> p u d", p=128))
                xb16 = b16p.tile([128, 2, D], BF16, tag="xb16")
                for u in range(2):
                    nc.vector.scalar_tensor_tensor(
                        out=prod, in0=xf[:, u, :], scalar=1.0, in1=wb,
                        op0=OP.mult, op1=OP.mult,
                        accum_out=alog[:, 2 * c + u:2 * c + u + 1])
                    if c < PREFIX_CHUNKS:
                        nc.scalar.activation(
                            out=xb16[:, u, :], in_=xf[:, u, :], func=AF.Copy)
                    else:
                        nc.gpsimd.tensor_copy(
                            out=xb16[:, u, :], in_=xf[:, u, :])
                    for h in range(2):
                        nc.tensor.matmul(
                            ps_xsum[:, h * 512:(h + 1) * 512],
                            ones16,
                            xb16[:, u, h * 512:(h + 1) * 512],
                            start=(c == 0 and u == 0),
                            stop=(c == NCHUNK - 1 and u == 1))
                    if c < PREFIX_CHUNKS:
                        g, sub = (0, 2 * c + u) if c < 2 else (1, u)
                        nc.sync.dma_start_transpose(
                            out=xsT[g][:, sub, :, :], in_=xb16[:, u, :])

            # ---------- ys load/cast/transpose ----------
            yf = xfp.tile([128, 2, D], F32, tag="xf")
            nc.sync.dma_start(
                out=yf, in_=ys[b].rearrange("(u p) d -> p u d", p=128))
            yb16 = b16p.tile([128, 2, D], BF16, tag="xb16")
            ysT = perb.tile([128, 2, 8, 128], BF16, tag="ysT")
            for u in range(2):
                nc.scalar.activation(out=yb16[:, u, :], in_=yf[:, u, :],
                                     func=AF.Copy)
                nc.sync.dma_start_transpose(out=ysT[:, u, :, :],
                                            in_=yb16[:, u, :])

            # ---------- proj + scores, per prefix group ----------
            xprojT = perb.tile([128, 8, W], BF16, tag="xprojT")   # [e%128, e//128, t]
            scores = perb.tile([128, 2, W], F32, tag="scores")    # [i%128, i//128, t]
            for g, (t0, tn) in enumerate([(0, 512), (512, 256)]):
                for e in range(8):
                    pp = psA.tile([128, 512], F32, tag="psA")
                    for k in range(8):
                        nc.tensor.matmul(
                            pp[:, :tn], wp16[:, k, e * 128:(e + 1) * 128],
                            xsT[g][:, :, k, :],
                            start=(k == 0), stop=(k == 7))
                    nc.scalar.activation(
                        out=xprojT[:, e, t0:t0 + tn], in_=pp[:, :tn],
                        func=AF.Identity, bias=bpT[:, e:e + 1])
                for i in range(2):
                    ps = psA.tile([128, 512], F32, tag="psA")
                    for k in range(8):
                        nc.tensor.matmul(
                            ps[:, :tn], ysT[:, i, k, :],
                            xprojT[:, k, t0:t0 + tn],
                            start=(k == 0), stop=(k == 7))
                    nc.scalar.activation(
                        out=scores[:, i, t0:t0 + tn], in_=ps[:, :tn],
                        func=AF.Copy, scale=float(SCALE))

            # xs_proj value-layout [t, e]: transpose back per (tblk, eblk)
            xproj_te = perb.tile([128, 6, 8, 128], BF16, tag="xte")
            for j in range(6):
                for e in range(8):
                    nc.sync.dma_start_transpose(
                        out=xproj_te[:, j, e, :],
                        in_=xprojT[:, e, j * 128:(j + 1) * 128])

            # ---------- alphas -> masked -> cumsum row ----------
            alpha_s = small.tile([128, 16], F32, tag="alpha_s")
            nc.scalar.activation(out=alpha_s, in_=alog, func=AF.Sigmoid,
                                 bias=bob)
            emask = small.tile([128, 16], F32, tag="emask")
            nc.vector.tensor_scalar(
                out=emask, in0=it_t, scalar1=xlen_bc[b], scalar2=None,
                op0=OP.is_lt)
            alpha_m = small.tile([128, 16], F32, tag="alpha_m")
            nc.vector.tensor_mul(alpha_m, alpha_s, emask)

            ps_at = psT.tile([16, 128], F32, tag="psT")
            nc.tensor.transpose(ps_at, alpha_m, idn)
            alphaT = small.tile([16, 128], F32, tag="alphaT")
            nc.vector.tensor_copy(alphaT, ps_at)
            nc.sync.dma_start(out=o_alpha[b, :].rearrange("(c p) -> c p", p=128),
                              in_=alphaT)
            crow = rows.tile([1, T], F32, tag="crow")
            nc.gpsimd.dma_start(out=crow, in_=alphaT)
            nc.vector.tensor_tensor_scan(
                out=crow, data0=crow, data1=crow, initial=0.0,
                op0=OP.add, op1=OP.bypass)
            nc.sync.dma_start(out=o_csum[b, :][None, :], in_=crow)
            nc.sync.dma_start(out=o_tok[b, :][None, :], in_=crow[:, T - 1:T])

            # masked csum row: +1e9 beyond xs_len, broadcast across partitions
            mrowt = rows.tile([1, W], F32, tag="mrowt")
            nc.vector.tensor_scalar(
                out=mrowt, in0=it_row[:, 0:W],
                scalar1=xlen_s[b],
                scalar2=1e9, op0=OP.is_ge, op1=OP.mult)
            nc.vector.tensor_add(mrowt, mrowt, crow[:, 0:W])
            nc.gpsimd.dma_start(out=mrow_dram[b, :][None, :], in_=mrowt)
            mbc = perb.tile([128, W], F32, tag="mbc")
            nc.gpsimd.dma_start(
                out=mbc, in_=mrow_dram[b, :].partition_broadcast(128))

            # ---------- fallback row: mean_t xs_proj ----------
            xsum_sb = rows.tile([1, D], F32, tag="rows")
            nc.scalar.activation(out=xsum_sb, in_=ps_xsum, func=AF.Copy,
                                 scale=1.0 / T)
            ps_xt = psT.tile([128, 8], F32, tag="psT")
            for k in range(8):
                nc.tensor.matmul(
                    ps_xt[:, k:k + 1], xsum_sb[:, k * 128:(k + 1) * 128],
                    one11_f, start=True, stop=True)
            xsumT16 = small.tile([128, 8], BF16, tag="xsumT16")
            nc.scalar.activation(out=xsumT16, in_=ps_xt, func=AF.Copy)
            ps_fb = psT.tile([1, D], F32, tag="psT")
            for h in range(2):
                for k in range(8):
                    nc.tensor.matmul(
                        ps_fb[:, h * 512:(h + 1) * 512],
                        xsumT16[:, k:k + 1],
                        wp16[:, k, h * 512:(h + 1) * 512],
                        start=(k == 0), stop=False)
                nc.tensor.matmul(
                    ps_fb[:, h * 512:(h + 1) * 512], one11_16,
                    bp_row16[:, h * 512:(h + 1) * 512],
                    start=False, stop=True)
            fb_row = rows.tile([1, D], F32, tag="rows")
            nc.vector.tensor_copy(fb_row, ps_fb)
            nc.gpsimd.dma_start(out=fb_dram[b, :], in_=fb_row)
            fb_bc = perb.tile([128, D], F32, tag="fb_bc")
            nc.gpsimd.dma_start(
                out=fb_bc, in_=fb_dram[b, :].partition_broadcast(128))

            # ---------- attention per query tile ----------
            for q in range(2):
                pen = outp.tile([128, W], F32, tag="pen")
                nc.vector.tensor_scalar(
                    out=pen, in0=mbc, scalar1=t_thr[:, q:q + 1],
                    scalar2=-1e9, op0=OP.is_ge, op1=OP.mult)
                msc = outp.tile([128, W], F32, tag="msc")
                nc.vector.tensor_add(msc, scores[:, q, :], pen)
                negmax = small.tile([128, 1], F32, tag="negmax")
                nc.vector.tensor_reduce(
                    out=negmax, in_=msc, axis=mybir.AxisListType.X,
                    op=OP.max, negate=True)
                probs = outp.tile([128, W], BF16, tag="probs")
                rowsum = small.tile([128, 1], F32, tag="rowsum")
                nc.scalar.activation(out=probs, in_=msc, func=AF.Exp,
                                     bias=negmax, accum_out=rowsum)
                recip = small.tile([128, 1], F32, tag="recip")
                nc.vector.reciprocal(recip, rowsum)

                probsT = outp.tile([128, 6, 128], BF16, tag="probsT")
                for j in range(6):
                    nc.sync.dma_start_transpose(
                        out=probsT[:, j, :], in_=probs[:, j * 128:(j + 1) * 128])

                qmask = small.tile([128, 1], F32, tag="qmask")
                nc.vector.tensor_scalar(
                    out=qmask, in0=t_ioq[:, q:q + 1], scalar1=ylen_bc[b],
                    scalar2=None, op0=OP.is_lt)
                nqmask = small.tile([128, 1], F32, tag="nqmask")
                nc.vector.tensor_scalar(
                    out=nqmask, in0=t_ioq[:, q:q + 1], scalar1=ylen_bc[b],
                    scalar2=None, op0=OP.is_ge)
                recm = small.tile([128, 1], F32, tag="recm")
                nc.vector.tensor_mul(recm, recip, qmask)
                fbm = outp.tile([128, D], F32, tag="fbm")
                nc.vector.tensor_scalar_mul(out=fbm, in0=fb_bc, scalar1=nqmask)

                emb = outp.tile([128, D], F32, tag="emb")
                for h in range(2):
                    pv = psPV.tile([128, 512], F32, tag="psPV")
                    for j in range(6):
                        nc.tensor.matmul(
                            pv, probsT[:, j, :],
                            xproj_te[:, j, 4 * h:4 * h + 4, :],
                            start=(j == 0), stop=(j == 5))
                    nc.vector.scalar_tensor_tensor(
                        out=emb[:, h * 512:(h + 1) * 512], in0=pv,
                        scalar=recm, in1=fbm[:, h * 512:(h + 1) * 512],
                        op0=OP.mult, op1=OP.add)
                nc.sync.dma_start(out=o_emb[b, q * 128:(q + 1) * 128, :],
                                  in_=emb)

    return nc


_SPLIT_OPS = {"DMACopy", "DmaTransposeAnt"}


def _split_multi_waits(bir: dict) -> dict:
    """The bass2jax walrus pass list keeps DMAs dynamic (PSEUDO_DMA_DIRECT2D),
    whose struct carries at most ONE sync-wait command.  Tile's scheduler can
    emit several waits on one DMA; split the extras into standalone
    EventSemaphore wait instructions (the exact shape wait_ge() produces) right
    before the DMA on the same engine stream — semantically identical."""
    for fn in bir["functions"]:
        for bb in fn["blocks"]:
            out = []
            for inst in bb["instructions"]:
                si = inst.get("sync_info") or {}
                waits = si.get("on_wait") or []
                if inst.get("opcode") not in ("EventSemaphore", "ISA") and len(waits) > 1:
                    for i, w in enumerate(waits[:-1]):
                        out.append({
                            "debug": inst.get("debug", 0),
                            "engine": inst["engine"],
                            "ins": [], "outs": [],
                            "name": f"{inst['name']}-w{i}",
                            "opcode": "EventSemaphore",
                            "sync_info": {"on_update": [], "on_wait": [w]},
                        })
                    si["on_wait"] = [waits[-1]]
                    inst["sync_info"] = si
                out.append(inst)
            bb["instructions"] = out
    return bir


def _patch_serialization(nc):
    import orjson
    orig = nc.to_json_bytes
    def patched():
        return orjson.dumps(_split_multi_waits(orjson.loads(orig())))
    nc.to_json_bytes = patched


_NC_CACHE = {}


def _get_nc():
    if "nc" not in _NC_CACHE:
        nc = _build_graph()
        _patch_serialization(nc)
        _NC_CACHE["nc"] = nc
    return _NC_CACHE["nc"]


def _np_reference_batch(xs, xs_len, ys, ys_len, W_out, b_out, W_proj, b_proj):
    """Numpy reference for one batch (host safety net)."""
    logits = xs.astype(np.float64) @ W_out.astype(np.float64) + b_out
    alphas = (1.0 / (1.0 + np.exp(-logits)))[:, 0]
    enc_mask = np.arange(T) < xs_len
    alphas = (alphas * enc_mask).astype(np.float32)
    csum = np.cumsum(alphas, dtype=np.float32)
    fire = np.arange(1, V + 1, dtype=np.float32)
    mask = csum[None, :] < fire[:, None]
    mask &= enc_mask[None, :]
    mask &= (np.arange(V) < ys_len)[:, None]
    xp = xs @ W_proj + b_proj
    sc = (ys @ xp.T) * SCALE
    sc = np.where(mask, sc, -1e9)
    sc = sc - sc.max(axis=-1, keepdims=True)
    e = np.exp(sc)
    attn = e / e.sum(axis=-1, keepdims=True)
    emb = attn @ xp
    return emb.astype(np.float32), np.float32(alphas.sum()), alphas, csum


def kernel(xs, xs_lens, ys, ys_lens, W_out, b_out, W_proj, b_proj):
    xs = np.ascontiguousarray(np.asarray(xs, dtype=np.float32))
    ys = np.ascontiguousarray(np.asarray(ys, dtype=np.float32))
    W_out = np.ascontiguousarray(np.asarray(W_out, dtype=np.float32))
    b_out = np.ascontiguousarray(np.asarray(b_out, dtype=np.float32))
    W_proj = np.ascontiguousarray(np.asarray(W_proj, dtype=np.float32))
    b_proj = np.ascontiguousarray(np.asarray(b_proj, dtype=np.float32))
    xs_lens_i = np.asarray(xs_lens)
    ys_lens_i = np.asarray(ys_lens)

    nc = _get_nc()

    iota_t = (np.arange(16)[None, :] * 128
              + np.arange(128)[:, None]).astype(np.float32)
    iota_row = np.ascontiguousarray(np.arange(T, dtype=np.float32)[None, :])
    ii = (np.arange(2)[None, :] * 128 + np.arange(128)[:, None])
    thr_q = np.ascontiguousarray((ii + 1).astype(np.float32))
    iota_q = np.ascontiguousarray(ii.astype(np.float32))
    ident = np.eye(128, dtype=np.float32)
    onesc = np.ones((128, 1), np.float32)

    in_maps = []
    for c in range(NCORES):
        sl = slice(NB * c, NB * (c + 1))
        lens_f = np.stack([np.asarray(xs_lens_i[sl], dtype=np.float32),
                           np.asarray(ys_lens_i[sl], dtype=np.float32)], axis=1)
        in_maps.append({
            "xs": xs[sl], "ys": ys[sl],
            "w_out": W_out, "b_out": b_out,
            "w_proj": W_proj, "b_proj": b_proj,
            "lens_f": np.ascontiguousarray(lens_f),
            "iota_t": iota_t, "iota_row": iota_row,
            "thr_q": thr_q, "iota_q": iota_q,
            "ident": ident, "onesc": onesc,
        })

    res = run_bass_kernel_spmd(
        nc, in_maps, core_ids=list(range(NCORES)),
        trace=bool(os.environ.get("AIF_TRACE")))
    kernel.last_exec_time_ns = res.exec_time_ns
    kernel.last_profile_json = res.profile_json
    kernel.last_insts = res.instructions_and_trace

    emb = np.concatenate([r["o_emb"] for r in res.results], axis=0)
    alphas = np.concatenate([r["o_alpha"] for r in res.results], axis=0)
    csum = np.concatenate([r["o_csum"] for r in res.results], axis=0)
    tok = np.concatenate([r["o_tok"] for r in res.results], axis=0)[:, 0]

    # Host safety net: any batch whose attended prefix exceeds the static
    # window W (or degenerate alphas[0] == 1.0) is recomputed exactly.
    t_idx = np.arange(T)[None, :]
    L = ((csum < np.asarray(ys_lens_i, dtype=np.float32)[:, None])
         & (t_idx < np.asarray(xs_lens_i)[:, None])).sum(axis=1)
    bad = (L > W) | (csum[:, 0] >= 1.0)
    for g in np.nonzero(bad)[0]:
        e_g, t_g, a_g, c_g = _np_reference_batch(
            xs[g], int(xs_lens_i[g]), ys[g], int(ys_lens_i[g]),
            W_out, b_out, W_proj, b_proj)
        emb[g], tok[g], alphas[g], csum[g] = e_g, t_g, a_g, c_g

    return emb, tok, alphas, csum


# revision 19
# speedup vs baseline: 1.9146x; 1.9146x over previous
"""AIF (attention-with-integrate-and-fire) sparse attention kernel for 8 TRN2 cores.

Strategy: data-parallel over batch (B=16 -> 2 batches/core, no collectives).
Per batch the AIF mask is a per-query prefix span: query i attends frames t with
csum[t] < i+1 (csum monotone) so only a short prefix (~2*ys_len <= ~512 frames)
of the 2048 encoder frames ever receives attention.  The kernel:
  - streams all xs once (f32) computing alphas (DVE dot), xs column-sum
    (f32r matmul) and the masked cumsum (PE transpose + DVE scan),
  - projects only a static 768-frame prefix window through W_proj in bf16,
  - computes masked softmax attention over that window (penalty mask from
    broadcast csum vs per-partition query thresholds, exp+rowsum fused on ACT),
  - fills queries i >= ys_len with the uniform-attention fallback row
    mean_t(xs_proj) = (sum_t xs / T) @ W_proj + b_proj via an on-device select.
A host-side safety net recomputes (numpy) any batch whose true attended prefix
exceeds the 768-frame window — impossible for the reference input distribution
(would need mean(sigmoid) < 1/3 over 768 frames) but keeps the kernel correct
for arbitrary inputs.
"""

import os
import sys
from contextlib import ExitStack

import numpy as np

sys.path.insert(0, "/opt/trn_rl_repo")
sys.path.insert(0, "/opt/trn_rl_repo/concourse")

import concourse.bass as bass
import concourse.tile as tile
from concourse import mybir
from concourse.bass_utils import run_bass_kernel_spmd

B, T, V = 16, 2048, 256
D = 1024          # ENC_D == PRED_D
NB = 2            # batches per core
NCORES = 8
W = 768           # static attended-prefix window (frames)
NCHUNK = T // 256   # 8 streaming chunks of 256 frames
PREFIX_CHUNKS = W // 256  # 3
SCALE = 1.0 / np.sqrt(D)
F32 = mybir.dt.float32
F32R = mybir.dt.float32r
BF16 = mybir.dt.bfloat16
AF = mybir.ActivationFunctionType
OP = mybir.AluOpType


def _build_graph():
    nc = bass.Bass()

    xs = nc.declare_dram_parameter("xs", [NB, T, D], F32, isOutput=False)
    ys = nc.declare_dram_parameter("ys", [NB, V, D], F32, isOutput=False)
    w_out = nc.declare_dram_parameter("w_out", [D, 1], F32, isOutput=False)
    b_out = nc.declare_dram_parameter("b_out", [1], F32, isOutput=False)
    w_proj = nc.declare_dram_parameter("w_proj", [D, D], F32, isOutput=False)
    b_proj = nc.declare_dram_parameter("b_proj", [D], F32, isOutput=False)
    lens_f = nc.declare_dram_parameter("lens_f", [NB, 2], F32, isOutput=False)  # [xs_len, ys_len]
    iota_t = nc.declare_dram_parameter("iota_t", [128, 16], F32, isOutput=False)    # t = c*128+p
    iota_row = nc.declare_dram_parameter("iota_row", [1, T], F32, isOutput=False)
    thr_q = nc.declare_dram_parameter("thr_q", [128, 2], F32, isOutput=False)       # i+1
    iota_q = nc.declare_dram_parameter("iota_q", [128, 2], F32, isOutput=False)     # i
    ident = nc.declare_dram_parameter("ident", [128, 128], F32, isOutput=False)
    onesc = nc.declare_dram_parameter("onesc", [128, 1], F32, isOutput=False)       # ones column

    mrow_dram = nc.dram_tensor("mrow_dram", [NB, W], F32)
    fb_dram = nc.dram_tensor("fb_dram", [NB, D], F32)

    o_emb = nc.declare_dram_parameter("o_emb", [NB, V, D], F32, isOutput=True)
    o_alpha = nc.declare_dram_parameter("o_alpha", [NB, T], F32, isOutput=True)
    o_csum = nc.declare_dram_parameter("o_csum", [NB, T], F32, isOutput=True)
    o_tok = nc.declare_dram_parameter("o_tok", [NB, 1], F32, isOutput=True)

    with tile.TileContext(nc) as tc, ExitStack() as ctx:
        singles = ctx.enter_context(tc.tile_pool(name="singles", bufs=1))
        xfp = ctx.enter_context(tc.tile_pool(name="xfp", bufs=3))
        b16p = ctx.enter_context(tc.tile_pool(name="b16p", bufs=2))
        xtp = ctx.enter_context(tc.tile_pool(name="xtp", bufs=1))
        perb = ctx.enter_context(tc.tile_pool(name="perb", bufs=1))
        small = ctx.enter_context(tc.tile_pool(name="small", bufs=4))
        scr = ctx.enter_context(tc.tile_pool(name="scr", bufs=1))
        rows = ctx.enter_context(tc.tile_pool(name="rows", bufs=2))
        outp = ctx.enter_context(tc.tile_pool(name="outp", bufs=2))
        psA = ctx.enter_context(tc.tile_pool(name="psA", bufs=3, space="PSUM"))
        psPV = ctx.enter_context(tc.tile_pool(name="psPV", bufs=2, space="PSUM"))
        psX = ctx.enter_context(tc.tile_pool(name="psX", bufs=1, space="PSUM"))
        psT = ctx.enter_context(tc.tile_pool(name="psT", bufs=1, space="PSUM"))

        # ---- constants / weights staged once ----
        wb = singles.tile([128, D], F32)          # W_out broadcast to all partitions
        nc.gpsimd.dma_start(out=wb, in_=w_out[:, 0].partition_broadcast(128))
        bob = singles.tile([128, 1], F32)         # b_out broadcast
        nc.gpsimd.dma_start(out=bob, in_=b_out[0:1].partition_broadcast(128))
        bpT = singles.tile([128, 8], F32)         # b_proj as [e%128, e//128]
        nc.gpsimd.dma_start(out=bpT, in_=bass.AP(
            tensor=b_proj.ap().tensor, offset=0, ap=[[1, 128], [128, 8]]))
        bp_row = rows.tile([1, D], F32, tag="rows")
        nc.gpsimd.dma_start(out=bp_row, in_=b_proj[None, :])
        bp_row16 = singles.tile([1, D], BF16)
        nc.scalar.activation(out=bp_row16, in_=bp_row, func=AF.Copy)
        one11_16 = singles.tile([1, 1], BF16)
        nc.vector.memset(one11_16, 1.0)
        one11_f = singles.tile([1, 1], F32)
        nc.vector.memset(one11_f, 1.0)

        it_t = singles.tile([128, 16], F32)
        nc.gpsimd.dma_start(out=it_t, in_=iota_t[:, :])
        it_row = singles.tile([1, T], F32)
        nc.gpsimd.dma_start(out=it_row, in_=iota_row[:, :])
        t_thr = singles.tile([128, 2], F32)
        nc.gpsimd.dma_start(out=t_thr, in_=thr_q[:, :])
        t_ioq = singles.tile([128, 2], F32)
        nc.gpsimd.dma_start(out=t_ioq, in_=iota_q[:, :])
        idn = singles.tile([128, 128], F32)
        nc.gpsimd.dma_start(out=idn, in_=ident[:, :])
        ones16 = singles.tile([128, 1], BF16)
        nc.vector.memset(ones16, 1.0)

        # W_proj: load f32 slabs through the xf ring, cast to resident bf16
        wp16 = singles.tile([128, 8, D], BF16)    # [d%128, d//128, e]
        for k in range(8):
            wslab = xfp.tile([128, D], F32, tag="xf")
            nc.sync.dma_start(out=wslab, in_=w_proj[k * 128:(k + 1) * 128, :])
            nc.scalar.activation(out=wp16[:, k, :], in_=wslab, func=AF.Copy)

        # per-batch broadcast of lens (128,1) plus (1,1) scalars
        xlen_bc, ylen_bc, xlen_s = [], [], []
        for b in range(NB):
            xb = small.tile([128, 1], F32, tag="lxb")
            nc.gpsimd.dma_start(
                out=xb, in_=lens_f[b, 0:1].partition_broadcast(128))
            yb = small.tile([128, 1], F32, tag="lyb")
            nc.gpsimd.dma_start(
                out=yb, in_=lens_f[b, 1:2].partition_broadcast(128))
            xs_ = small.tile([1, 1], F32, tag="lxs")
            nc.gpsimd.dma_start(out=xs_, in_=lens_f[b:b + 1, 0:1])
            xlen_bc.append(xb)
            ylen_bc.append(yb)
            xlen_s.append(xs_)

        prod = scr.tile([128, D], BF16)           # dummy full-product output

        for b in range(NB):
            alog = small.tile([128, 16], F32, tag="alog")
            ps_xsum = psX.tile([1, D], F32, tag="xsum")
            # transposed bf16 xs prefix, per 512/256 group
            xsT0 = xtp.tile([128, 4, 8, 128], BF16, tag="xsT0")
            xsT1 = xtp.tile([128, 2, 8, 128], BF16, tag="xsT1")
            xsT = [xsT0, xsT1]

            # ---------- phase A: stream xs ----------
            for c in range(NCHUNK):
                xf = xfp.tile([128, 2, D], F32, tag="xf")
                nc.sync.dma_start(
                    out=xf,
                    in_=xs[b, c * 256:(c + 1) * 256, :].rearrange(
                        "(u p) d -> p u d", p=128))
                xb16 = b16p.tile([128, 2, D], BF16, tag="xb16")
                nc.scalar.activation(out=xb16, in_=xf, func=AF.Copy)
                for u in range(2):
                    nc.vector.scalar_tensor_tensor(
                        out=prod, in0=xf[:, u, :], scalar=1.0, in1=wb,
                        op0=OP.mult, op1=OP.mult,
                        accum_out=alog[:, 2 * c + u:2 * c + u + 1])
                    for h in range(2):
                        nc.tensor.matmul(
                            ps_xsum[:, h * 512:(h + 1) * 512],
                            ones16,
                            xb16[:, u, h * 512:(h + 1) * 512],
                            start=(c == 0 and u == 0),
                            stop=(c == NCHUNK - 1 and u == 1))
                    if c < PREFIX_CHUNKS:
                        g, sub = (0, 2 * c + u) if c < 2 else (1, u)
                        nc.sync.dma_start_transpose(
                            out=xsT[g][:, sub, :, :], in_=xb16[:, u, :])

            # ---------- ys load/cast/transpose ----------
            yf = xfp.tile([128, 2, D], F32, tag="xf")
            nc.sync.dma_start(
                out=yf, in_=ys[b].rearrange("(u p) d -> p u d", p=128))
            yb16 = b16p.tile([128, 2, D], BF16, tag="xb16")
            ysT = perb.tile([128, 2, 8, 128], BF16, tag="ysT")
            nc.scalar.activation(out=yb16, in_=yf, func=AF.Copy)
            for u in range(2):
                nc.sync.dma_start_transpose(out=ysT[:, u, :, :],
                                            in_=yb16[:, u, :])

            # ---------- proj + scores, per prefix group ----------
            xprojT = perb.tile([128, 8, W], BF16, tag="xprojT")   # [e%128, e//128, t]
            scores = perb.tile([128, 2, W], F32, tag="scores")    # [i%128, i//128, t]
            # value-layout [t%128, e//128, t//128, e%128]
            xproj_te = perb.tile([128, 8, 6, 128], BF16, tag="xte")
            for e in range(8):
                pp0 = psA.tile([128, 512], F32, tag="psA")
                pp1 = psA.tile([128, 512], F32, tag="psA")
                for k in range(8):
                    nc.tensor.matmul(
                        pp0, wp16[:, k, e * 128:(e + 1) * 128],
                        xsT[0][:, :, k, :], start=(k == 0), stop=(k == 7))
                    nc.tensor.matmul(
                        pp1[:, :256], wp16[:, k, e * 128:(e + 1) * 128],
                        xsT[1][:, :, k, :], start=(k == 0), stop=(k == 7))
                nc.scalar.activation(
                    out=xprojT[:, e, 0:512], in_=pp0,
                    func=AF.Identity, bias=bpT[:, e:e + 1])
                nc.scalar.activation(
                    out=xprojT[:, e, 512:W], in_=pp1[:, :256],
                    func=AF.Identity, bias=bpT[:, e:e + 1])
                nc.sync.dma_start_transpose(
                    out=xproj_te[:, e, :, :], in_=xprojT[:, e, :])
            for g, (t0, tn) in enumerate([(0, 512), (512, 256)]):
                for i in range(2):
                    ps = psA.tile([128, 512], F32, tag="psA")
                    for k in range(8):
                        nc.tensor.matmul(
                            ps[:, :tn], ysT[:, i, k, :],
                            xprojT[:, k, t0:t0 + tn],
                            start=(k == 0), stop=(k == 7))
                    nc.scalar.activation(
                        out=scores[:, i, t0:t0 + tn], in_=ps[:, :tn],
                        func=AF.Copy, scale=float(SCALE))

            # ---------- alphas -> masked -> cumsum row ----------
            alpha_s = small.tile([128, 16], F32, tag="alpha_s")
            nc.scalar.activation(out=alpha_s, in_=alog, func=AF.Sigmoid,
                                 bias=bob)
            emask = small.tile([128, 16], F32, tag="emask")
            nc.vector.tensor_scalar(
                out=emask, in0=it_t, scalar1=xlen_bc[b], scalar2=None,
                op0=OP.is_lt)
            alpha_m = small.tile([128, 16], F32, tag="alpha_m")
            nc.vector.tensor_mul(alpha_m, alpha_s, emask)

            ps_at = psT.tile([16, 128], F32, tag="psT")
            nc.tensor.transpose(ps_at, alpha_m, idn)
            alphaT = small.tile([16, 128], F32, tag="alphaT")
            nc.vector.tensor_copy(alphaT, ps_at)
            nc.sync.dma_start(out=o_alpha[b, :].rearrange("(c p) -> c p", p=128),
                              in_=alphaT)
            crow = rows.tile([1, T], F32, tag="crow")
            nc.gpsimd.dma_start(out=crow, in_=alphaT)
            nc.vector.tensor_tensor_scan(
                out=crow, data0=crow, data1=crow, initial=0.0,
                op0=OP.add, op1=OP.bypass)
            nc.sync.dma_start(out=o_csum[b, :][None, :], in_=crow)
            nc.sync.dma_start(out=o_tok[b, :][None, :], in_=crow[:, T - 1:T])

            # masked csum row: +1e9 beyond xs_len, broadcast across partitions
            mrowt = rows.tile([1, W], F32, tag="mrowt")
            nc.vector.tensor_scalar(
                out=mrowt, in0=it_row[:, 0:W],
                scalar1=xlen_s[b],
                scalar2=1e9, op0=OP.is_ge, op1=OP.mult)
            nc.vector.tensor_add(mrowt, mrowt, crow[:, 0:W])
            nc.gpsimd.dma_start(out=mrow_dram[b, :][None, :], in_=mrowt)
            mbc = perb.tile([128, W], F32, tag="mbc")
            nc.gpsimd.dma_start(
                out=mbc, in_=mrow_dram[b, :].partition_broadcast(128))

            # ---------- fallback row: mean_t xs_proj ----------
            xsum_sb = rows.tile([1, D], F32, tag="rows")
            nc.scalar.activation(out=xsum_sb, in_=ps_xsum, func=AF.Copy,
                                 scale=1.0 / T)
            ps_xt = psT.tile([128, 8], F32, tag="psT")
            for k in range(8):
                nc.tensor.matmul(
                    ps_xt[:, k:k + 1], xsum_sb[:, k * 128:(k + 1) * 128],
                    one11_f, start=True, stop=True)
            xsumT16 = small.tile([128, 8], BF16, tag="xsumT16")
            nc.scalar.activation(out=xsumT16, in_=ps_xt, func=AF.Copy)
            fb_row = rows.tile([1, D], F32, tag="rows")
            for h in range(2):
                ps_fb = psT.tile([1, 512], F32, tag="psT")
                for k in range(8):
                    nc.tensor.matmul(
                        ps_fb, xsumT16[:, k:k + 1],
                        wp16[:, k, h * 512:(h + 1) * 512],
                        start=(k == 0), stop=False)
                nc.tensor.matmul(
                    ps_fb, one11_16,
                    bp_row16[:, h * 512:(h + 1) * 512],
                    start=False, stop=True)
                nc.vector.tensor_copy(fb_row[:, h * 512:(h + 1) * 512], ps_fb)
            nc.gpsimd.dma_start(out=fb_dram[b, :], in_=fb_row)
            fb_bc = perb.tile([128, D], F32, tag="fb_bc")
            nc.gpsimd.dma_start(
                out=fb_bc, in_=fb_dram[b, :].partition_broadcast(128))

            # ---------- attention per query tile ----------
            for q in range(2):
                pen = outp.tile([128, W], F32, tag="pen")
                nc.vector.tensor_scalar(
                    out=pen, in0=mbc, scalar1=t_thr[:, q:q + 1],
                    scalar2=-1e9, op0=OP.is_ge, op1=OP.mult)
                msc = outp.tile([128, W], F32, tag="msc")
                nc.vector.tensor_add(msc, scores[:, q, :], pen)
                negmax = small.tile([128, 1], F32, tag="negmax")
                nc.vector.tensor_reduce(
                    out=negmax, in_=msc, axis=mybir.AxisListType.X,
                    op=OP.max, negate=True)
                probs = outp.tile([128, W], BF16, tag="probs")
                rowsum = small.tile([128, 1], F32, tag="rowsum")
                nc.scalar.activation(out=probs, in_=msc, func=AF.Exp,
                                     bias=negmax, accum_out=rowsum)
                recip = small.tile([128, 1], F32, tag="recip")
                nc.vector.reciprocal(recip, rowsum)

                probsT = outp.tile([128, 6, 128], BF16, tag="probsT")
                nc.sync.dma_start_transpose(out=probsT, in_=probs)

                qmask = small.tile([128, 1], F32, tag="qmask")
                nc.vector.tensor_scalar(
                    out=qmask, in0=t_ioq[:, q:q + 1], scalar1=ylen_bc[b],
                    scalar2=None, op0=OP.is_lt)
                nqmask = small.tile([128, 1], F32, tag="nqmask")
                nc.vector.tensor_scalar(
                    out=nqmask, in0=t_ioq[:, q:q + 1], scalar1=ylen_bc[b],
                    scalar2=None, op0=OP.is_ge)
                recm = small.tile([128, 1], F32, tag="recm")
                nc.vector.tensor_mul(recm, recip, qmask)
                fbm = outp.tile([128, D], F32, tag="fbm")
                nc.vector.tensor_scalar_mul(out=fbm, in0=fb_bc, scalar1=nqmask)

                emb = outp.tile([128, D], F32, tag="emb")
                for h in range(2):
                    pv = psPV.tile([128, 512], F32, tag="psPV")
                    for j in range(6):
                        nc.tensor.matmul(
                            pv, probsT[:, j, :],
                            xproj_te[:, 4 * h:4 * h + 4, j, :],
                            start=(j == 0), stop=(j == 5))
                    nc.vector.scalar_tensor_tensor(
                        out=emb[:, h * 512:(h + 1) * 512], in0=pv,
                        scalar=recm, in1=fbm[:, h * 512:(h + 1) * 512],
                        op0=OP.mult, op1=OP.add)
                nc.scalar.dma_start(out=o_emb[b, q * 128:(q + 1) * 128, :],
                                    in_=emb)

    return nc


_SPLIT_OPS = {"DMACopy", "DmaTransposeAnt"}


def _split_multi_waits(bir: dict) -> dict:
    """The bass2jax walrus pass list keeps DMAs dynamic (PSEUDO_DMA_DIRECT2D),
    whose struct carries at most ONE sync-wait command.  Tile's scheduler can
    emit several waits on one DMA; split the extras into standalone
    EventSemaphore wait instructions (the exact shape wait_ge() produces) right
    before the DMA on the same engine stream — semantically identical."""
    for fn in bir["functions"]:
        for bb in fn["blocks"]:
            out = []
            for inst in bb["instructions"]:
                si = inst.get("sync_info") or {}
                waits = si.get("on_wait") or []
                if inst.get("opcode") not in ("EventSemaphore", "ISA") and len(waits) > 1:
                    for i, w in enumerate(waits[:-1]):
                        out.append({
                            "debug": inst.get("debug", 0),
                            "engine": inst["engine"],
                            "ins": [], "outs": [],
                            "name": f"{inst['name']}-w{i}",
                            "opcode": "EventSemaphore",
                            "sync_info": {"on_update": [], "on_wait": [w]},
                        })
                    si["on_wait"] = [waits[-1]]
                    inst["sync_info"] = si
                out.append(inst)
            bb["instructions"] = out
    return bir


def _patch_serialization(nc):
    import orjson
    orig = nc.to_json_bytes
    def patched():
        return orjson.dumps(_split_multi_waits(orjson.loads(orig())))
    nc.to_json_bytes = patched


_NC_CACHE = {}


def _get_nc():
    if "nc" not in _NC_CACHE:
        nc = _build_graph()
        _patch_serialization(nc)
        _NC_CACHE["nc"] = nc
    return _NC_CACHE["nc"]


def _np_reference_batch(xs, xs_len, ys, ys_len, W_out, b_out, W_proj, b_proj):
    """Numpy reference for one batch (host safety net)."""
    logits = xs.astype(np.float64) @ W_out.astype(np.float64) + b_out
    alphas = (1.0 / (1.0 + np.exp(-logits)))[:, 0]
    enc_mask = np.arange(T) < xs_len
    alphas = (alphas * enc_mask).astype(np.float32)
    csum = np.cumsum(alphas, dtype=np.float32)
    fire = np.arange(1, V + 1, dtype=np.float32)
    mask = csum[None, :] < fire[:, None]
    mask &= enc_mask[None, :]
    mask &= (np.arange(V) < ys_len)[:, None]
    xp = xs @ W_proj + b_proj
    sc = (ys @ xp.T) * SCALE
    sc = np.where(mask, sc, -1e9)
    sc = sc - sc.max(axis=-1, keepdims=True)
    e = np.exp(sc)
    attn = e / e.sum(axis=-1, keepdims=True)
    emb = attn @ xp
    return emb.astype(np.float32), np.float32(alphas.sum()), alphas, csum


def kernel(xs, xs_lens, ys, ys_lens, W_out, b_out, W_proj, b_proj):
    xs = np.ascontiguousarray(np.asarray(xs, dtype=np.float32))
    ys = np.ascontiguousarray(np.asarray(ys, dtype=np.float32))
    W_out = np.ascontiguousarray(np.asarray(W_out, dtype=np.float32))
    b_out = np.ascontiguousarray(np.asarray(b_out, dtype=np.float32))
    W_proj = np.ascontiguousarray(np.asarray(W_proj, dtype=np.float32))
    b_proj = np.ascontiguousarray(np.asarray(b_proj, dtype=np.float32))
    xs_lens_i = np.asarray(xs_lens)
    ys_lens_i = np.asarray(ys_lens)

    nc = _get_nc()

    iota_t = (np.arange(16)[None, :] * 128
              + np.arange(128)[:, None]).astype(np.float32)
    iota_row = np.ascontiguousarray(np.arange(T, dtype=np.float32)[None, :])
    ii = (np.arange(2)[None, :] * 128 + np.arange(128)[:, None])
    thr_q = np.ascontiguousarray((ii + 1).astype(np.float32))
    iota_q = np.ascontiguousarray(ii.astype(np.float32))
    ident = np.eye(128, dtype=np.float32)
    onesc = np.ones((128, 1), np.float32)

    in_maps = []
    for c in range(NCORES):
        sl = slice(NB * c, NB * (c + 1))
        lens_f = np.stack([np.asarray(xs_lens_i[sl], dtype=np.float32),
                           np.asarray(ys_lens_i[sl], dtype=np.float32)], axis=1)
        in_maps.append({
            "xs": xs[sl], "ys": ys[sl],
            "w_out": W_out, "b_out": b_out,
            "w_proj": W_proj, "b_proj": b_proj,
            "lens_f": np.ascontiguousarray(lens_f),
            "iota_t": iota_t, "iota_row": iota_row,
            "thr_q": thr_q, "iota_q": iota_q,
            "ident": ident, "onesc": onesc,
        })

    res = run_bass_kernel_spmd(
        nc, in_maps, core_ids=list(range(NCORES)),
        trace=bool(os.environ.get("AIF_TRACE")))
    kernel.last_exec_time_ns = res.exec_time_ns
    kernel.last_profile_json = res.profile_json
    kernel.last_insts = res.instructions_and_trace

    emb = np.concatenate([r["o_emb"] for r in res.results], axis=0)
    alphas = np.concatenate([r["o_alpha"] for r in res.results], axis=0)
    csum = np.concatenate([r["o_csum"] for r in res.results], axis=0)
    tok = np.concatenate([r["o_tok"] for r in res.results], axis=0)[:, 0]

    # Host safety net: any batch whose attended prefix exceeds the static
    # window W (or degenerate alphas[0] == 1.0) is recomputed exactly.
    t_idx = np.arange(T)[None, :]
    L = ((csum < np.asarray(ys_lens_i, dtype=np.float32)[:, None])
         & (t_idx < np.asarray(xs_lens_i)[:, None])).sum(axis=1)
    bad = (L > W) | (csum[:, 0] >= 1.0)
    for g in np.nonzero(bad)[0]:
        e_g, t_g, a_g, c_g = _np_reference_batch(
            xs[g], int(xs_lens_i[g]), ys[g], int(ys_lens_i[g]),
            W_out, b_out, W_proj, b_proj)
        emb[g], tok[g], alphas[g], csum[g] = e_g, t_g, a_g, c_g

    return emb, tok, alphas, csum
